# revision 22
# baseline (speedup 1.0000x reference)
"""ExpertLinear (dense MoE blend) Trainium2 kernel — expert-sharded.

y[b,o] = sum_k ew[b,k] * (x[b,:] @ W[k,o,:]) + sum_k ew[b,k] * bias[k,o]

Sharding: one expert per core (E == 8 == NCORES). Each core computes its
expert's full GEMM z_c = x @ W[c].T for ALL B rows, scales by ew[:, c] on
eviction, and writes a bf16 partial; the host sums the 8 partials and adds
the (tiny) bias term. This reads each expert's weights exactly once
chip-wide: per-core HBM traffic is ~4 MB (vs ~18.5 MB for data-parallel),
turning a DMA-bound kernel into a PE-bound one (~13.7 us of bf16 matmul).

Layout/precision:
  - Host packs, per core, an interleaved stream of 8 i-chunks; chunk n =
    [wT tile n | xT tile n] as one [128, 1536] bf16 DMA, so the PE's
    i-major loop starts after one chunk lands and each matmul group needs
    exactly ONE sync wait (this walrus build rejects >1 wait/instruction).
  - ew column arrives fp32 [128, 4]; scaling happens on eviction via
    per-partition tensor_scalar_mul (DVE) / activation Copy scale (ACT),
    split across both engines so the eviction tail halves.
  - PSUM: all 8 banks hold the [512, 1024] fp32 partial (4 b-chunks x 2
    o-halves); i-major accumulation, single eviction per bank.
  - A few zero-matmuls at the start keep the PE busy during the DMA
    lead-in so the HAM clock-gate un-throttles (1.2 -> 2.4 GHz) sooner.
"""

import numpy as np

B, E, IN, OUT = 512, 8, 1024, 1024
NCORES = 8
P = 128
NIT = IN // P      # 8 i-tiles (contraction chunks)
BT = B // P        # 4 b-chunks (output partition tiles)
NH = OUT // 512    # 2 o-halves (PSUM bank free-dim limit)
CW = OUT + B  # 1536 cols per i-tile: wT tile (1024) + xT tile (512)
XOFF = OUT          # x region offset inside an i-tile block
N_DUMMY = 7
EWPAD = 16          # extra bf16 cols on chunk 0 carrying the ew column
# i-tile 0 is split so the PE can start on [wt0-half0 | x0] alone; the
# second half rides with i-tile 1. Remaining i-tiles in pairs. 5 input
# chunks + 3 outputs = 8 DMAs = one per DMAHW lane.
CHUNKS = [(2, 4), (4, 6), (6, 8)]

_compiled = None


def _patch_drain_split():
    """The walrus build in this container rejects any instruction carrying
    more than one sync wait, including the kernel-tail Drain that
    TileContext emits with one wait per active semaphore. Split it into a
    sequence of single-wait drains (sequencer-FIFO keeps them ordered;
    the set of waits is identical)."""
    import concourse.tile as tile_mod

    if getattr(tile_mod.TileContext, "_drain_split_patched", False):
        return
    from concourse.tile_sem_assignment import N_PROCS
    from concourse.vector_clock import ScopedClock, VectorClock

    def _drain_and_barrier(self, tick_clock, wait_clock):
        gc = tick_clock.global_clock
        for p in range(N_PROCS):
            t = gc[p]
            if t <= 0:
                continue
            ticks = [0] * N_PROCS
            ticks[p] = t
            di = self.nc.sync.drain()
            wait_clock.add_sem_waits(
                di.ins, ScopedClock({None: VectorClock(ticks)})
            )
        self.nc.all_engine_barrier()
        assert self.sems is not None
        popped = self.nc._tile_sem_poison_stack.pop()
        assert popped is self._sem_poison
        # bookkeeping of clear_and_free_semaphores WITHOUT emitting the
        # gpsimd clear + trailing barrier: the NEFF-level teardown wipes
        # the whole sem space anyway, and nothing in this program runs
        # after the barrier above -- saves ~1 us of kernel tail
        sem_nums = [s.num for s in self.sems.allocated().values()]
        self.nc._state.prepend_free_semaphores(sem_nums)
        for poison_set in self.nc._tile_sem_poison_stack:
            poison_set.update(sem_nums)

    tile_mod.TileContext._drain_and_barrier = _drain_and_barrier
    tile_mod.TileContext._drain_split_patched = True


def _build():
    import concourse.bass as bass
    import concourse.mybir as mybir
    import concourse.tile as tile

    _patch_drain_split()

    f32 = mybir.dt.float32
    bf16 = mybir.dt.bfloat16
    Copy = mybir.ActivationFunctionType.Copy

    nc = bass.Bass()
    # chunk 0 split: wx0a = [wt0 cols 0:512 | xT tile 0 | ew], wx0b = rest
    wx0a_d = nc.dram_tensor(
        "wx0a", [P, 512 + B + EWPAD], bf16, kind="ExternalInput"
    )
    wx0b_d = nc.dram_tensor("wx0b", [P, 512 + CW], bf16, kind="ExternalInput")
    wxr_d = nc.dram_tensor(
        "wxr", [(NIT - 2) * P, CW], bf16, kind="ExternalInput"
    )
    yv_d = nc.dram_tensor("yv", [P, BT * 512], bf16, kind="ExternalOutput")
    ya_d = nc.dram_tensor("ya", [P, BT * 512], bf16, kind="ExternalOutput")

    with tile.TileContext(nc) as tc:
        with (
            tc.tile_pool(name="sb", bufs=1) as sb,
            tc.tile_pool(name="ps", bufs=1, space="PSUM") as psp,
        ):
            ewt = sb.tile([P, BT], f32, name="ewt", tag="ewt")
            scr_v = sb.tile([P, 1], f32, name="scrv", tag="scrv")
            scr_s = sb.tile([1, BT], f32, name="scrs", tag="scrs")
            wx0a = sb.tile([P, 512 + B + EWPAD], bf16, name="wx0a", tag="wx0a")
            wx0b = sb.tile([P, 512 + CW], bf16, name="wx0b", tag="wx0b")
            wxs = [
                sb.tile([P, (e - s) * CW], bf16, name=f"wx{ci}", tag=f"wx{ci}")
                for ci, (s, e) in enumerate(CHUNKS)
            ]
            y_v = sb.tile([P, BT * 512], bf16, name="yv", tag="yv")
            y_a = sb.tile([P, BT * 512], bf16, name="ya", tag="ya")
            pss = [
                [
                    psp.tile([P, 512], f32, name=f"ps{t}{h}", tag=f"ps{t}{h}")
                    for h in range(NH)
                ]
                for t in range(BT)
            ]

            # HAM warmers: matmuls over (uninitialized) y_v keep the PE
            # array busy from engine-boot until the first chunk lands, so
            # the clock-gate reaches 8/8 before the real matmuls start.
            # Their garbage output lands in bank (0,0), which the real
            # group's start=True clears.
            for _ in range(N_DUMMY):
                nc.tensor.matmul(
                    pss[0][0][0:1, :], y_v[:, 0:1], y_v[:, 0:512],
                    start=True, stop=True, skip_group_check=True,
                )

            # exactly 8 HWDGE DMAs in the whole kernel -> each DMAHW lane
            # is used once, so no DMA ever needs a lane-recycle wait on
            # top of its data wait (single-wait limit). wx0 first so the
            # PE's first real group is gated only by it; ew is not needed
            # until eviction.
            nc.sync.dma_start(wx0a[:], wx0a_d[:])
            nc.sync.dma_start(wx0b[:], wx0b_d[:])
            for ci, (s, e) in enumerate(CHUNKS):
                src = wxr_d[(s - 2) * P:(e - 2) * P, :].rearrange(
                    "(n p) c -> p n c", p=P
                )
                dst = wxs[ci][:].rearrange("p (n c) -> p n c", n=e - s)
                nc.sync.dma_start(dst, src)

            # i-tile 0, half 0: gated only on the small wx0a DMA
            for t in range(BT):
                lhsT0 = wx0a[:, 512 + P * t:512 + P * (t + 1)]
                nc.tensor.matmul(
                    pss[t][0][:], lhsT0, wx0a[:, 0:512],
                    start=True, stop=False, skip_group_check=(t == 0),
                )
            # i-tile 0, half 1 (first 512 cols of wx0b)
            for t in range(BT):
                lhsT0 = wx0a[:, 512 + P * t:512 + P * (t + 1)]
                nc.tensor.matmul(
                    pss[t][1][:], lhsT0, wx0b[:, 0:512],
                    start=True, stop=False,
                )
            # i-tile 1 (rides in wx0b at offset 512)
            for t in range(BT):
                lhsT1 = wx0b[:, 512 + XOFF + P * t:512 + XOFF + P * (t + 1)]
                for h in range(NH):
                    nc.tensor.matmul(
                        pss[t][h][:], lhsT1,
                        wx0b[:, 512 + 512 * h:512 + 512 * (h + 1)],
                        start=False, stop=False,
                        skip_group_check=(t == 0 and h == 0),
                    )
            # remaining i-tiles: chunk-major so a group waits only on its
            # chunk's DMA; within a chunk, bank-major so banks finish
            # staggered in the last chunk and evictions pipeline behind
            # the PE instead of serializing after it.
            for ci, (s, e) in enumerate(CHUNKS):
                wx = wxs[ci]
                for t in range(BT):
                    for n in range(s, e):
                        off = (n - s) * CW
                        lhsT = wx[
                            :, off + XOFF + P * t:off + XOFF + P * (t + 1)
                        ]
                        for h in range(NH):
                            nc.tensor.matmul(
                                pss[t][h][:], lhsT,
                                wx[:, off + 512 * h:off + 512 * (h + 1)],
                                start=False,
                                stop=(n == e - 1 and ci == len(CHUNKS) - 1),
                                skip_group_check=(t == 0 and h == 0),
                            )

            # ew rides in chunk 0 as bf16; DVE upconverts it once (this
            # also absorbs the chunk-0 DMA wait for DVE), and the ACT
            # absorber reads the converted copy so real evictions carry
            # only their PE wait (single-wait limit)
            nc.vector.tensor_copy(ewt[:], wx0a[:, 512 + B:512 + B + BT])
            # absorber: reads ewt through the tensor_scalar ptr path so the
            # real DVE evicts don't carry a second (DVE-seq) wait
            nc.vector.tensor_scalar_mul(scr_v[:], wx0a[:, 0:1], ewt[:, 0:1])
            nc.scalar.activation(scr_s[:], ewt[0:1, :], Copy)

            # evict: y[b,:] = ps[b,:] * ew[b]; DVE takes h=0, ACT h=1
            for t in range(BT):
                sc = ewt[:, t:t + 1]
                nc.vector.tensor_scalar_mul(
                    y_v[:, t * 512:(t + 1) * 512], pss[t][0][:], sc
                )
                nc.scalar.activation(
                    y_a[:, t * 512:(t + 1) * 512], pss[t][1][:], Copy, scale=sc
                )
            nc.sync.dma_start(ya_d[:, 0:1536], y_a[:, 0:1536])
            nc.sync.dma_start(yv_d[:], y_v[:])
            nc.sync.dma_start(ya_d[:, 1536:2048], y_a[:, 1536:2048])

    return nc


def _get_compiled():
    global _compiled
    if _compiled is None:
        _compiled = _build()
    return _compiled


_pack_cache = None


def _make_in_maps(x, expert_weights, weight, bias):
    global _pack_cache
    import ml_dtypes

    bf16 = ml_dtypes.bfloat16
    if _pack_cache is None or _pack_cache[0] is not weight:
        w = np.asarray(weight, dtype=np.float32)
        wx0as, wx0bs, wxrs = [], [], []
        for c in range(NCORES):
            wT = w[c].T.reshape(NIT, P, OUT).astype(bf16)  # [p,o]=W[c,o,128n+p]
            a0 = np.zeros((P, 512 + B + EWPAD), dtype=bf16)
            a0[:, :512] = wT[0, :, :512]
            b0 = np.zeros((P, 512 + CW), dtype=bf16)
            b0[:, :512] = wT[0, :, 512:]
            b0[:, 512:512 + OUT] = wT[1]
            ar = np.zeros((NIT - 2, P, CW), dtype=bf16)
            ar[:, :, :OUT] = wT[2:]
            wx0as.append(a0)
            wx0bs.append(b0)
            wxrs.append(ar)
        _pack_cache = (weight, wx0as, wx0bs, wxrs)
    _, wx0as, wx0bs, wxrs = _pack_cache

    x = np.asarray(x, dtype=np.float32)
    ew = np.asarray(expert_weights, dtype=np.float32)
    # xT tile n: [p, b] = x[b, 128n+p]
    xTb = x.T.reshape(NIT, P, B).astype(bf16)
    in_maps = []
    for c in range(NCORES):
        wx0as[c][:, 512:512 + B] = xTb[0]
        wx0as[c][:, 512 + B:512 + B + BT] = (
            ew[:, c].reshape(BT, P).T.astype(bf16)
        )
        wx0bs[c][:, 512 + XOFF:512 + CW] = xTb[1]
        wxrs[c][:, :, XOFF:] = xTb[2:]
        in_maps.append({
            "wx0a": wx0as[c],
            "wx0b": wx0bs[c],
            "wxr": wxrs[c].reshape((NIT - 2) * P, CW),
        })
    return in_maps


def kernel(x, expert_weights, weight, bias, _trace=False):
    from concourse.bass_utils import run_bass_kernel_spmd

    nc = _get_compiled()
    in_maps = _make_in_maps(x, expert_weights, weight, bias)
    res = run_bass_kernel_spmd(
        nc, in_maps, core_ids=list(range(NCORES)), trace=_trace
    )
    acc = np.zeros((B, OUT), dtype=np.float32)
    for r in res.results:
        # yv[p, t*512+j] = y[128t+p, j]; ya[p, t*512+j] = y[128t+p, 512+j]
        yv = np.asarray(r["yv"], dtype=np.float32).reshape(P, BT, 512)
        ya = np.asarray(r["ya"], dtype=np.float32).reshape(P, BT, 512)
        acc[:, :512] += yv.transpose(1, 0, 2).reshape(B, 512)
        acc[:, 512:] += ya.transpose(1, 0, 2).reshape(B, 512)
    ew = np.asarray(expert_weights, dtype=np.float32)
    b = np.asarray(bias, dtype=np.float32)
    y = acc + ew @ b
    if _trace:
        return y, res
    return y


# revision 23
# speedup vs baseline: 1.1348x; 1.1348x over previous
"""ExpertLinear (dense MoE blend) Trainium2 kernel — expert-sharded.

y[b,o] = sum_k ew[b,k] * (x[b,:] @ W[k,o,:]) + sum_k ew[b,k] * bias[k,o]

Sharding: one expert per core (E == 8 == NCORES). Each core computes its
expert's full GEMM z_c = x @ W[c].T for ALL B rows, scales by ew[:, c] on
eviction, and writes a bf16 partial; the host sums the 8 partials and adds
the (tiny) bias term. This reads each expert's weights exactly once
chip-wide: per-core HBM traffic is ~4 MB (vs ~18.5 MB for data-parallel),
turning a DMA-bound kernel into a PE-bound one (~13.7 us of bf16 matmul).

Layout/precision:
  - Host packs, per core, an interleaved stream of 8 i-chunks; chunk n =
    [wT tile n | xT tile n] as one [128, 1536] bf16 DMA, so the PE's
    i-major loop starts after one chunk lands and each matmul group needs
    exactly ONE sync wait (this walrus build rejects >1 wait/instruction).
  - ew column arrives fp32 [128, 4]; scaling happens on eviction via
    per-partition tensor_scalar_mul (DVE) / activation Copy scale (ACT),
    split across both engines so the eviction tail halves.
  - PSUM: all 8 banks hold the [512, 1024] fp32 partial (4 b-chunks x 2
    o-halves); i-major accumulation, single eviction per bank.
  - A few zero-matmuls at the start keep the PE busy during the DMA
    lead-in so the HAM clock-gate un-throttles (1.2 -> 2.4 GHz) sooner.
"""

import numpy as np

B, E, IN, OUT = 512, 8, 1024, 1024
NCORES = 8
P = 128
NIT = IN // P      # 8 i-tiles (contraction chunks)
BT = B // P        # 4 b-chunks (output partition tiles)
NH = OUT // 512    # 2 o-halves (PSUM bank free-dim limit)
CW = OUT + B  # 1536 cols per i-tile: wT tile (1024) + xT tile (512)
XOFF = OUT          # x region offset inside an i-tile block
N_DUMMY = 9
EWPAD = 16          # extra bf16 cols on chunk 0 carrying the ew column
# i-tile ranges per DMA chunk: fine-grained early chunks keep every
# chunk's completion semaphore >=1.1 us ahead of the PE even when all 8
# cores contend for HBM. 5 input chunks + 3 outputs = 8 DMAs = one per
# DMAHW lane.
CHUNKS = [(0, 1), (1, 2), (2, 3), (3, 5), (5, 8)]

_compiled = None


def _patch_drain_split():
    """The walrus build in this container rejects any instruction carrying
    more than one sync wait, including the kernel-tail Drain that
    TileContext emits with one wait per active semaphore. Split it into a
    sequence of single-wait drains (sequencer-FIFO keeps them ordered;
    the set of waits is identical)."""
    import concourse.tile as tile_mod

    if getattr(tile_mod.TileContext, "_drain_split_patched", False):
        return
    from concourse.tile_sem_assignment import N_PROCS
    from concourse.vector_clock import ScopedClock, VectorClock

    def _drain_and_barrier(self, tick_clock, wait_clock):
        gc = tick_clock.global_clock
        for p in range(N_PROCS):
            t = gc[p]
            if t <= 0:
                continue
            ticks = [0] * N_PROCS
            ticks[p] = t
            di = self.nc.sync.drain()
            wait_clock.add_sem_waits(
                di.ins, ScopedClock({None: VectorClock(ticks)})
            )
        self.nc.all_engine_barrier()
        assert self.sems is not None
        popped = self.nc._tile_sem_poison_stack.pop()
        assert popped is self._sem_poison
        # bookkeeping of clear_and_free_semaphores WITHOUT emitting the
        # gpsimd clear + trailing barrier: the NEFF-level teardown wipes
        # the whole sem space anyway, and nothing in this program runs
        # after the barrier above -- saves ~1 us of kernel tail
        sem_nums = [s.num for s in self.sems.allocated().values()]
        self.nc._state.prepend_free_semaphores(sem_nums)
        for poison_set in self.nc._tile_sem_poison_stack:
            poison_set.update(sem_nums)

    tile_mod.TileContext._drain_and_barrier = _drain_and_barrier
    tile_mod.TileContext._drain_split_patched = True


def _build():
    import concourse.bass as bass
    import concourse.mybir as mybir
    import concourse.tile as tile

    _patch_drain_split()

    f32 = mybir.dt.float32
    bf16 = mybir.dt.bfloat16
    Copy = mybir.ActivationFunctionType.Copy

    nc = bass.Bass()
    # chunk 0 carries [wt0 | xT tile 0 | ew]
    wx0_d = nc.dram_tensor("wx0", [P, CW + EWPAD], bf16, kind="ExternalInput")
    wxr_d = nc.dram_tensor(
        "wxr", [(NIT - 1) * P, CW], bf16, kind="ExternalInput"
    )
    yv_d = nc.dram_tensor("yv", [P, BT * 512], bf16, kind="ExternalOutput")
    ya_d = nc.dram_tensor("ya", [P, BT * 512], bf16, kind="ExternalOutput")

    with tile.TileContext(nc) as tc:
        with (
            tc.tile_pool(name="sb", bufs=1) as sb,
            tc.tile_pool(name="ps", bufs=1, space="PSUM") as psp,
        ):
            ewt = sb.tile([P, BT], f32, name="ewt", tag="ewt")
            scr_v = sb.tile([P, 1], f32, name="scrv", tag="scrv")
            scr_s = sb.tile([1, BT], f32, name="scrs", tag="scrs")
            wx0 = sb.tile([P, CW + EWPAD], bf16, name="wx0", tag="wx0")
            wxs = [
                sb.tile([P, (e - s) * CW], bf16, name=f"wx{ci}", tag=f"wx{ci}")
                for ci, (s, e) in enumerate(CHUNKS[1:], start=1)
            ]
            y_v = sb.tile([P, BT * 512], bf16, name="yv", tag="yv")
            y_a = sb.tile([P, BT * 512], bf16, name="ya", tag="ya")
            pss = [
                [
                    psp.tile([P, 512], f32, name=f"ps{t}{h}", tag=f"ps{t}{h}")
                    for h in range(NH)
                ]
                for t in range(BT)
            ]

            # HAM warmers: matmuls over (uninitialized) y_v keep the PE
            # array busy from engine-boot until the first chunk lands, so
            # the clock-gate reaches 8/8 before the real matmuls start.
            # Their garbage output lands in bank (0,0), which the real
            # group's start=True clears.
            for _ in range(N_DUMMY):
                nc.tensor.matmul(
                    pss[0][0][0:1, :], y_v[:, 0:1], y_v[:, 0:512],
                    start=True, stop=True, skip_group_check=True,
                )

            # exactly 8 HWDGE DMAs in the whole kernel -> each DMAHW lane
            # is used once, so no DMA ever needs a lane-recycle wait on
            # top of its data wait (single-wait limit). wx0 first so the
            # PE's first real group is gated only by it; ew is not needed
            # until eviction.
            nc.sync.dma_start(wx0[:], wx0_d[:])
            for ci, (s, e) in enumerate(CHUNKS[1:], start=1):
                src = wxr_d[(s - 1) * P:(e - 1) * P, :].rearrange(
                    "(n p) c -> p n c", p=P
                )
                dst = wxs[ci - 1][:].rearrange("p (n c) -> p n c", n=e - s)
                nc.sync.dma_start(dst, src)

            # i-tile 0 from wx0
            for t in range(BT):
                lhsT0 = wx0[:, XOFF + P * t:XOFF + P * (t + 1)]
                for h in range(NH):
                    nc.tensor.matmul(
                        pss[t][h][:], lhsT0, wx0[:, 512 * h:512 * (h + 1)],
                        start=True, stop=False,
                        skip_group_check=(t == 0 and h == 0),
                    )
            # remaining i-tiles: chunk-major so a group waits only on its
            # chunk's DMA; within a chunk, bank-major so banks finish
            # staggered in the last chunk and evictions pipeline behind
            # the PE instead of serializing after it.
            for ci, (s, e) in enumerate(CHUNKS[1:], start=1):
                wx = wxs[ci - 1]
                for t in range(BT):
                    for n in range(s, e):
                        off = (n - s) * CW
                        lhsT = wx[
                            :, off + XOFF + P * t:off + XOFF + P * (t + 1)
                        ]
                        for h in range(NH):
                            nc.tensor.matmul(
                                pss[t][h][:], lhsT,
                                wx[:, off + 512 * h:off + 512 * (h + 1)],
                                start=False,
                                stop=(n == e - 1 and ci == len(CHUNKS) - 1),
                                skip_group_check=(t == 0 and h == 0),
                            )

            # ew rides in chunk 0 as bf16; DVE upconverts it once (this
            # also absorbs the chunk-0 DMA wait for DVE), and the ACT
            # absorber reads the converted copy so real evictions carry
            # only their PE wait (single-wait limit)
            nc.vector.tensor_copy(ewt[:], wx0[:, CW:CW + BT])
            # absorber: reads ewt through the tensor_scalar ptr path so the
            # real DVE evicts don't carry a second (DVE-seq) wait
            nc.vector.tensor_scalar_mul(scr_v[:], wx0[:, 0:1], ewt[:, 0:1])
            nc.scalar.activation(scr_s[:], ewt[0:1, :], Copy)

            # evict: y[b,:] = ps[b,:] * ew[b]; DVE takes h=0, ACT h=1
            for t in range(BT):
                sc = ewt[:, t:t + 1]
                nc.vector.tensor_scalar_mul(
                    y_v[:, t * 512:(t + 1) * 512], pss[t][0][:], sc
                )
                nc.scalar.activation(
                    y_a[:, t * 512:(t + 1) * 512], pss[t][1][:], Copy, scale=sc
                )
            nc.sync.dma_start(ya_d[:, 0:1536], y_a[:, 0:1536])
            nc.sync.dma_start(yv_d[:], y_v[:])
            nc.sync.dma_start(ya_d[:, 1536:2048], y_a[:, 1536:2048])

    return nc


def _get_compiled():
    global _compiled
    if _compiled is None:
        _compiled = _build()
    return _compiled


_pack_cache = None


def _make_in_maps(x, expert_weights, weight, bias):
    global _pack_cache
    import ml_dtypes

    bf16 = ml_dtypes.bfloat16
    if _pack_cache is None or _pack_cache[0] is not weight:
        w = np.asarray(weight, dtype=np.float32)
        wx0s, wxrs = [], []
        for c in range(NCORES):
            wT = w[c].T.reshape(NIT, P, OUT).astype(bf16)  # [p,o]=W[c,o,128n+p]
            a0 = np.zeros((P, CW + EWPAD), dtype=bf16)
            a0[:, :OUT] = wT[0]
            ar = np.zeros((NIT - 1, P, CW), dtype=bf16)
            ar[:, :, :OUT] = wT[1:]
            wx0s.append(a0)
            wxrs.append(ar)
        _pack_cache = (weight, wx0s, wxrs)
    _, wx0s, wxrs = _pack_cache

    x = np.asarray(x, dtype=np.float32)
    ew = np.asarray(expert_weights, dtype=np.float32)
    # xT tile n: [p, b] = x[b, 128n+p]
    xTb = x.T.reshape(NIT, P, B).astype(bf16)
    in_maps = []
    for c in range(NCORES):
        wx0s[c][:, XOFF:XOFF + B] = xTb[0]
        wx0s[c][:, CW:CW + BT] = ew[:, c].reshape(BT, P).T.astype(bf16)
        wxrs[c][:, :, XOFF:] = xTb[1:]
        in_maps.append({
            "wx0": wx0s[c],
            "wxr": wxrs[c].reshape((NIT - 1) * P, CW),
        })
    return in_maps


def kernel(x, expert_weights, weight, bias, _trace=False):
    from concourse.bass_utils import run_bass_kernel_spmd

    nc = _get_compiled()
    in_maps = _make_in_maps(x, expert_weights, weight, bias)
    res = run_bass_kernel_spmd(
        nc, in_maps, core_ids=list(range(NCORES)), trace=_trace
    )
    acc = np.zeros((B, OUT), dtype=np.float32)
    for r in res.results:
        # yv[p, t*512+j] = y[128t+p, j]; ya[p, t*512+j] = y[128t+p, 512+j]
        yv = np.asarray(r["yv"], dtype=np.float32).reshape(P, BT, 512)
        ya = np.asarray(r["ya"], dtype=np.float32).reshape(P, BT, 512)
        acc[:, :512] += yv.transpose(1, 0, 2).reshape(B, 512)
        acc[:, 512:] += ya.transpose(1, 0, 2).reshape(B, 512)
    ew = np.asarray(expert_weights, dtype=np.float32)
    b = np.asarray(bias, dtype=np.float32)
    y = acc + ew @ b
    if _trace:
        return y, res
    return y
